# revision 24
# baseline (speedup 1.0000x reference)
"""LIF neuron scan kernel for Trainium2, sharded over 8 NeuronCores.

Device recurrence, ONE custom DVE instruction per time step (f32):
    u_t = I_t + 0.95 * (u_{t-1} * (u_{t-1} < 1))
computed in-place over the input tile (state = previous u slice), via a
registered custom DVE op (4 ALU stages).  The mask-multiply is exact, so
this matches the fused form u = round(round(0.95*v)+I).

The fused decay differs from the reference's (v - v/20) by <= ~4e-6
over the whole trajectory (measured with synced resets), so the spike
raster can only flip where u lands within that distance of threshold.
The device emits one fp8-e4m3 plane sig = sigmoid(8192*(u-1)) (ACT
engine): bytes decode monotonically in u, sigma > 0.5 <=> u > 1, and any
u within 3.8e-6 of threshold maps within 0.008 of sigma=0.5 -- far
inside one fp8 quantum (0.0625) -- so the host flags bytes near 0.5 as
suspects and recomputes those rows bit-exactly with the reference
formula (~1e-5 of rows).

Sharding: batch dim B=131072 split into 8 contiguous blocks of 16384
rows. Per core the block is laid out time-major as [128 partitions, 400
steps, 128 neurons] so each step is one [128,128] SBUF slice and DMA
chunks are per-partition contiguous.
"""

import os
import numpy as np

import concourse.bacc as bacc
import concourse.mybir as mybir
from concourse.tile import TileContext
from concourse.bass_utils import run_bass_kernel_spmd
from concourse.mybir import AluOpType as Op

B, L = 131072, 400
NCORES = 8
RPC = B // NCORES      # rows (neurons) per core
P = 128                # SBUF partitions
J = RPC // P           # neurons per partition = 128 (one step = [P, J] slice)

# Chunk schedule: geometric ramp so each chunk's input DMA lands just in
# time while DVE chews the previous ones; small tail chunks so the final
# ACT/DMA drain hides under compute. Sums to L.
CHUNKS = [8, 16, 32] + [64] * 5 + [16, 8]
assert sum(CHUNKS) == L

G = int(os.environ.get("BASS_LIF_G", "2"))        # interleaved groups
PLANES = os.environ.get("BASS_LIF_PLANES", "sig8")
JG = J // G

DECAY_MUL95 = 0.95
DECAY_MUL05 = 0.05
TH = 1.0
EPS = 1e-4            # sign2 band half-width
KSIG = 8192.0         # sig8 sigmoid sharpness

_nc_cache = None
_lif_op = None


def _register_lif_op():
    """Register the fused LIF-step custom DVE op (idempotent):
    out = in0 + (in1 * (in1 < s1)) * s0
    """
    global _lif_op
    if _lif_op is not None:
        return _lif_op
    import concourse.dve_ops as dve_ops
    from concourse.dve_spec import Spec, Src0, Src1, C0, C1, lower
    from concourse.dve_spec import _has_src1
    from concourse.dve_uop import DveOpSpec

    name = "LIF_STEP_ANT"
    for op in dve_ops.OPS:
        if op.name == name:
            _lif_op = op
            return op
    body = Src0 + (Src1 * (Src1 < C1)) * C0
    spec = Spec(
        body=body,
        reference=lambda in0, in1, s0, s1, imm2: (
            in0 + (in1 * (in1 < s1).astype(np.float32)) * np.float32(s0)
        ).astype(np.float32),
    )
    sha = {}
    for ver in ("v3", "v4"):
        sha[ver] = DveOpSpec(
            name=name, opcode=0x1F, uops=lower(spec, ver=ver),
            rd1_en=_has_src1(spec),
        ).sha(ver)
    op = dve_ops.DveOp(name, spec, subdim=False, uops_sha=sha)
    dve_ops.OPS.append(op)
    dve_ops.CUSTOM_DVE_SPECS[name] = spec
    row = dve_ops._CUSTOM_DVE_ROW_BASE + len(dve_ops.OPS) - 1
    assert row < 0x20, "custom-DVE opcode rows exhausted"
    dve_ops._SUB_OPCODE_FOR_NAME[name] = row
    _lif_op = op
    return op


def _build():
    nc = bacc.Bacc(None, target_bir_lowering=False)
    X = nc.dram_tensor("X", [P, L * J], mybir.dt.float32, kind="ExternalInput")
    if PLANES == "sig8":
        S8 = nc.dram_tensor("S8", [P, L * J], mybir.dt.float8e4, kind="ExternalOutput")
    else:
        A = nc.dram_tensor("A", [P, L * J], mybir.dt.uint8, kind="ExternalOutput")
        Bp = nc.dram_tensor("Bq", [P, L * J], mybir.dt.uint8, kind="ExternalOutput")
    lif = _register_lif_op()

    with TileContext(nc) as tc:
        with (
            tc.tile_pool(name="state", bufs=1) as state_pool,
            tc.tile_pool(name="io", bufs=5) as io_pool,
            tc.tile_pool(name="pl", bufs=3) as pl_pool,
        ):
            if PLANES == "sig8":
                bsg = state_pool.tile([P, 1], mybir.dt.float32, name="bsg")
                nc.vector.memset(bsg[:], -KSIG)
            else:
                blo = state_pool.tile([P, 1], mybir.dt.float32, name="blo")
                bhi = state_pool.tile([P, 1], mybir.dt.float32, name="bhi")
                nc.vector.memset(blo[:], float(EPS - TH))
                nc.vector.memset(bhi[:], float(-EPS - TH))
            prev_tile, prev_t = None, 0
            t0 = 0
            for ch, TC in enumerate(CHUNKS):
                base = t0 * J
                t0 += TC
                xin = io_pool.tile([P, TC * J], mybir.dt.float32, name="xin")
                nc.sync.dma_start(xin[:], X[:, base : base + TC * J])
                for t in range(TC):
                    if prev_tile is None:
                        # v init is 0, so u_0 = I_0: the DMA'd input slice
                        # already is u_0 -- skip the first step's op.
                        prev_tile, prev_t = xin, 0
                        continue
                    sls = [
                        slice(t * J + g * JG, t * J + (g + 1) * JG) for g in range(G)
                    ]
                    psls = [
                        slice(prev_t * J + g * JG, prev_t * J + (g + 1) * JG)
                        for g in range(G)
                    ]
                    for g in range(G):
                        nc.vector._custom_dve(
                            lif,
                            out=xin[:, sls[g]],
                            in0=xin[:, sls[g]],
                            in1=prev_tile[:, psls[g]],
                            s0=DECAY_MUL95,
                            s1=TH,
                        )
                    prev_tile, prev_t = xin, t
                if PLANES == "sig8":
                    # sig = sigmoid(KSIG*(u-1)) -> fp8: monotone byte code of u;
                    # bytes near 0.5 (|u-1| <~ 4e-5) are the host-repair band.
                    ps = pl_pool.tile([P, TC * J], mybir.dt.float8e4, name="ps")
                    nc.scalar.activation(
                        ps[:], xin[:], mybir.ActivationFunctionType.Sigmoid,
                        bias=bsg[:], scale=KSIG,
                    )
                    nc.scalar.dma_start(S8[:, base : base + TC * J], ps[:])
                else:
                    pa = pl_pool.tile([P, TC * J], mybir.dt.uint8, name="pa")
                    pb = pl_pool.tile([P, TC * J], mybir.dt.uint8, name="pb")
                    nc.scalar.activation(
                        pa[:], xin[:], mybir.ActivationFunctionType.Sign,
                        bias=blo[:], scale=1.0,
                    )
                    nc.scalar.activation(
                        pb[:], xin[:], mybir.ActivationFunctionType.Sign,
                        bias=bhi[:], scale=1.0,
                    )
                    nc.scalar.dma_start(A[:, base : base + TC * J], pa[:])
                    nc.scalar.dma_start(Bp[:, base : base + TC * J], pb[:])
    nc.compile()
    return nc


def _get_nc():
    global _nc_cache
    if _nc_cache is None:
        _nc_cache = _build()
    return _nc_cache


def _shard(I):
    # Per-core host transposes run in parallel (numpy releases the GIL
    # during the strided copies).
    from concurrent.futures import ThreadPoolExecutor

    def one(c):
        Ic = I[c * RPC : (c + 1) * RPC]                    # [RPC, L]
        Xc = Ic.reshape(P, J, L).transpose(0, 2, 1)        # [P, L, J] time-major
        return {"X": np.ascontiguousarray(Xc).reshape(P, L * J)}

    with ThreadPoolExecutor(NCORES) as ex:
        return list(ex.map(one, range(NCORES)))


def _unshard_plane(results, key):
    from concurrent.futures import ThreadPoolExecutor

    out = np.empty((B, L), np.uint8)

    def one(c):
        r = np.asarray(results[c][key])
        if r.dtype != np.uint8:
            r = r.view(np.uint8) if r.dtype.itemsize == 1 else r.astype(np.uint8)
        Sc = r.reshape(P, L, J).transpose(0, 2, 1)         # [P, J, L]
        out[c * RPC : (c + 1) * RPC] = Sc.reshape(RPC, L)

    with ThreadPoolExecutor(NCORES) as ex:
        list(ex.map(one, range(NCORES)))
    return out


def _f8e4m3_lut():
    """byte -> float32 value of fp8 e4m3 (bias 7)."""
    b = np.arange(256, dtype=np.uint32)
    sign = np.where(b >> 7, -1.0, 1.0)
    e = (b >> 3) & 0xF
    m = b & 0x7
    val = np.where(
        e == 0,
        (m / 8.0) * 2.0 ** (-6),
        (1.0 + m / 8.0) * (2.0 ** (e.astype(np.int32) - 7)),
    )
    return (sign * val).astype(np.float32)


def _decode(I, results):
    f32 = np.float32
    if PLANES == "sig8":
        raw = _unshard_plane(results, "S8")
        val = _f8e4m3_lut()[raw]
        spikes = val > f32(0.5)
        suspect = np.abs(val - f32(0.5)) <= f32(0.05)
    else:
        pa = _unshard_plane(results, "A")
        pb = _unshard_plane(results, "Bq")
        spikes = pa == 1
        suspect = spikes & (pb != 1)
    rows = np.nonzero(suspect.any(axis=1))[0]
    out = spikes.astype(f32)
    if rows.size:
        # Bit-exact reference recurrence for the suspect rows only.
        Ir = I[rows]
        v = np.zeros(rows.size, f32)
        for t in range(L):
            u = ((v - v * f32(DECAY_MUL05)) + Ir[:, t]).astype(f32)
            s = u >= f32(TH)
            out[rows, t] = s
            v = np.where(s, f32(0.0), u)
    return out


def kernel(I, _trace=False):
    I = np.ascontiguousarray(np.asarray(I), dtype=np.float32)
    assert I.shape == (B, L), I.shape
    nc = _get_nc()
    br = run_bass_kernel_spmd(nc, _shard(I), core_ids=list(range(NCORES)), trace=_trace)
    out = _decode(I, br.results)
    if _trace:
        return out, br
    return out


# revision 25
# speedup vs baseline: 1.0328x; 1.0328x over previous
"""LIF neuron scan kernel for Trainium2, sharded over 8 NeuronCores.

Device recurrence, ONE custom DVE instruction per time step (f32):
    u_t = I_t + 0.95 * (u_{t-1} * (u_{t-1} < 1))
computed in-place over the input tile (state = previous u slice), via a
registered custom DVE op (4 ALU stages).  The mask-multiply is exact, so
this matches the fused form u = round(round(0.95*v)+I).

The fused decay differs from the reference's (v - v/20) by <= ~4e-6
over the whole trajectory (measured with synced resets), so the spike
raster can only flip where u lands within that distance of threshold.
The device emits one fp8-e4m3 plane sig = sigmoid(8192*(u-1)) (ACT
engine): bytes decode monotonically in u, sigma > 0.5 <=> u > 1, and any
u within 3.8e-6 of threshold maps within 0.008 of sigma=0.5 -- far
inside one fp8 quantum (0.0625) -- so the host flags bytes near 0.5 as
suspects and recomputes those rows bit-exactly with the reference
formula (~1e-5 of rows).

Sharding: batch dim B=131072 split into 8 contiguous blocks of 16384
rows. Per core the block is laid out time-major as [128 partitions, 400
steps, 128 neurons] so each step is one [128,128] SBUF slice and DMA
chunks are per-partition contiguous.
"""

import os
import numpy as np

import concourse.bacc as bacc
import concourse.mybir as mybir
from concourse.tile import TileContext
from concourse.bass_utils import run_bass_kernel_spmd
from concourse.mybir import AluOpType as Op

B, L = 131072, 400
NCORES = 8
RPC = B // NCORES      # rows (neurons) per core
P = 128                # SBUF partitions
J = RPC // P           # neurons per partition = 128 (one step = [P, J] slice)

# Chunk schedule: geometric ramp so each chunk's input DMA lands just in
# time while DVE chews the previous ones; small tail chunks so the final
# ACT/DMA drain hides under compute. Sums to L.
CHUNKS = [8, 16, 32] + [64] * 4 + [40, 24, 16, 4, 4]
assert sum(CHUNKS) == L

G = int(os.environ.get("BASS_LIF_G", "2"))        # interleaved groups
PLANES = os.environ.get("BASS_LIF_PLANES", "sig8")
JG = J // G

DECAY_MUL95 = 0.95
DECAY_MUL05 = 0.05
TH = 1.0
EPS = 1e-4            # sign2 band half-width
KSIG = 8192.0         # sig8 sigmoid sharpness

_nc_cache = None
_lif_op = None


def _register_lif_op():
    """Register the fused LIF-step custom DVE op (idempotent):
    out = in0 + (in1 * (in1 < s1)) * s0
    """
    global _lif_op
    if _lif_op is not None:
        return _lif_op
    import concourse.dve_ops as dve_ops
    from concourse.dve_spec import Spec, Src0, Src1, C0, C1, lower
    from concourse.dve_spec import _has_src1
    from concourse.dve_uop import DveOpSpec

    name = "LIF_STEP_ANT"
    for op in dve_ops.OPS:
        if op.name == name:
            _lif_op = op
            return op
    body = Src0 + (Src1 * (Src1 < C1)) * C0
    spec = Spec(
        body=body,
        reference=lambda in0, in1, s0, s1, imm2: (
            in0 + (in1 * (in1 < s1).astype(np.float32)) * np.float32(s0)
        ).astype(np.float32),
    )
    sha = {}
    for ver in ("v3", "v4"):
        sha[ver] = DveOpSpec(
            name=name, opcode=0x1F, uops=lower(spec, ver=ver),
            rd1_en=_has_src1(spec),
        ).sha(ver)
    op = dve_ops.DveOp(name, spec, subdim=False, uops_sha=sha)
    dve_ops.OPS.append(op)
    dve_ops.CUSTOM_DVE_SPECS[name] = spec
    row = dve_ops._CUSTOM_DVE_ROW_BASE + len(dve_ops.OPS) - 1
    assert row < 0x20, "custom-DVE opcode rows exhausted"
    dve_ops._SUB_OPCODE_FOR_NAME[name] = row
    _lif_op = op
    return op


def _build():
    nc = bacc.Bacc(None, target_bir_lowering=False)
    X = nc.dram_tensor("X", [P, L * J], mybir.dt.float32, kind="ExternalInput")
    if PLANES == "sig8":
        S8 = nc.dram_tensor("S8", [P, L * J], mybir.dt.float8e4, kind="ExternalOutput")
    else:
        A = nc.dram_tensor("A", [P, L * J], mybir.dt.uint8, kind="ExternalOutput")
        Bp = nc.dram_tensor("Bq", [P, L * J], mybir.dt.uint8, kind="ExternalOutput")
    lif = _register_lif_op()

    with TileContext(nc) as tc:
        with (
            tc.tile_pool(name="state", bufs=1) as state_pool,
            tc.tile_pool(name="io", bufs=5) as io_pool,
            tc.tile_pool(name="pl", bufs=3) as pl_pool,
        ):
            if PLANES == "sig8":
                bsg = state_pool.tile([P, 1], mybir.dt.float32, name="bsg")
                nc.vector.memset(bsg[:], -KSIG)
            else:
                blo = state_pool.tile([P, 1], mybir.dt.float32, name="blo")
                bhi = state_pool.tile([P, 1], mybir.dt.float32, name="bhi")
                nc.vector.memset(blo[:], float(EPS - TH))
                nc.vector.memset(bhi[:], float(-EPS - TH))
            prev_tile, prev_t = None, 0
            t0 = 0
            for ch, TC in enumerate(CHUNKS):
                base = t0 * J
                t0 += TC
                xin = io_pool.tile([P, TC * J], mybir.dt.float32, name="xin")
                nc.sync.dma_start(xin[:], X[:, base : base + TC * J])
                for t in range(TC):
                    if prev_tile is None:
                        # v init is 0, so u_0 = I_0: the DMA'd input slice
                        # already is u_0 -- skip the first step's op.
                        prev_tile, prev_t = xin, 0
                        continue
                    sls = [
                        slice(t * J + g * JG, t * J + (g + 1) * JG) for g in range(G)
                    ]
                    psls = [
                        slice(prev_t * J + g * JG, prev_t * J + (g + 1) * JG)
                        for g in range(G)
                    ]
                    for g in range(G):
                        nc.vector._custom_dve(
                            lif,
                            out=xin[:, sls[g]],
                            in0=xin[:, sls[g]],
                            in1=prev_tile[:, psls[g]],
                            s0=DECAY_MUL95,
                            s1=TH,
                        )
                    prev_tile, prev_t = xin, t
                if PLANES == "sig8":
                    # sig = sigmoid(KSIG*(u-1)) -> fp8: monotone byte code of u;
                    # bytes near 0.5 (|u-1| <~ 4e-5) are the host-repair band.
                    ps = pl_pool.tile([P, TC * J], mybir.dt.float8e4, name="ps")
                    nc.scalar.activation(
                        ps[:], xin[:], mybir.ActivationFunctionType.Sigmoid,
                        bias=bsg[:], scale=KSIG,
                    )
                    nc.scalar.dma_start(S8[:, base : base + TC * J], ps[:])
                else:
                    pa = pl_pool.tile([P, TC * J], mybir.dt.uint8, name="pa")
                    pb = pl_pool.tile([P, TC * J], mybir.dt.uint8, name="pb")
                    nc.scalar.activation(
                        pa[:], xin[:], mybir.ActivationFunctionType.Sign,
                        bias=blo[:], scale=1.0,
                    )
                    nc.scalar.activation(
                        pb[:], xin[:], mybir.ActivationFunctionType.Sign,
                        bias=bhi[:], scale=1.0,
                    )
                    nc.scalar.dma_start(A[:, base : base + TC * J], pa[:])
                    nc.scalar.dma_start(Bp[:, base : base + TC * J], pb[:])
    nc.compile()
    return nc


def _get_nc():
    global _nc_cache
    if _nc_cache is None:
        _nc_cache = _build()
    return _nc_cache


def _shard(I):
    # Per-core host transposes run in parallel (numpy releases the GIL
    # during the strided copies).
    from concurrent.futures import ThreadPoolExecutor

    def one(c):
        Ic = I[c * RPC : (c + 1) * RPC]                    # [RPC, L]
        Xc = Ic.reshape(P, J, L).transpose(0, 2, 1)        # [P, L, J] time-major
        return {"X": np.ascontiguousarray(Xc).reshape(P, L * J)}

    with ThreadPoolExecutor(NCORES) as ex:
        return list(ex.map(one, range(NCORES)))


def _unshard_plane(results, key):
    from concurrent.futures import ThreadPoolExecutor

    out = np.empty((B, L), np.uint8)

    def one(c):
        r = np.asarray(results[c][key])
        if r.dtype != np.uint8:
            r = r.view(np.uint8) if r.dtype.itemsize == 1 else r.astype(np.uint8)
        Sc = r.reshape(P, L, J).transpose(0, 2, 1)         # [P, J, L]
        out[c * RPC : (c + 1) * RPC] = Sc.reshape(RPC, L)

    with ThreadPoolExecutor(NCORES) as ex:
        list(ex.map(one, range(NCORES)))
    return out


def _f8e4m3_lut():
    """byte -> float32 value of fp8 e4m3 (bias 7)."""
    b = np.arange(256, dtype=np.uint32)
    sign = np.where(b >> 7, -1.0, 1.0)
    e = (b >> 3) & 0xF
    m = b & 0x7
    val = np.where(
        e == 0,
        (m / 8.0) * 2.0 ** (-6),
        (1.0 + m / 8.0) * (2.0 ** (e.astype(np.int32) - 7)),
    )
    return (sign * val).astype(np.float32)


def _decode(I, results):
    f32 = np.float32
    if PLANES == "sig8":
        raw = _unshard_plane(results, "S8")
        val = _f8e4m3_lut()[raw]
        spikes = val > f32(0.5)
        suspect = np.abs(val - f32(0.5)) <= f32(0.05)
    else:
        pa = _unshard_plane(results, "A")
        pb = _unshard_plane(results, "Bq")
        spikes = pa == 1
        suspect = spikes & (pb != 1)
    rows = np.nonzero(suspect.any(axis=1))[0]
    out = spikes.astype(f32)
    if rows.size:
        # Bit-exact reference recurrence for the suspect rows only.
        Ir = I[rows]
        v = np.zeros(rows.size, f32)
        for t in range(L):
            u = ((v - v * f32(DECAY_MUL05)) + Ir[:, t]).astype(f32)
            s = u >= f32(TH)
            out[rows, t] = s
            v = np.where(s, f32(0.0), u)
    return out


def kernel(I, _trace=False):
    I = np.ascontiguousarray(np.asarray(I), dtype=np.float32)
    assert I.shape == (B, L), I.shape
    nc = _get_nc()
    br = run_bass_kernel_spmd(nc, _shard(I), core_ids=list(range(NCORES)), trace=_trace)
    out = _decode(I, br.results)
    if _trace:
        return out, br
    return out


# revision 26
# speedup vs baseline: 1.0331x; 1.0003x over previous
"""LIF neuron scan kernel for Trainium2, sharded over 8 NeuronCores.

Device recurrence, ONE custom DVE instruction per time step (f32):
    u_t = I_t + 0.95 * (u_{t-1} * (u_{t-1} < 1))
computed in-place over the input tile (state = previous u slice), via a
registered custom DVE op (4 ALU stages).  The mask-multiply is exact, so
this matches the fused form u = round(round(0.95*v)+I).

The fused decay differs from the reference's (v - v/20) by <= ~4e-6
over the whole trajectory (measured with synced resets), so the spike
raster can only flip where u lands within that distance of threshold.
The device emits one fp8-e4m3 plane sig = sigmoid(8192*(u-1)) (ACT
engine): bytes decode monotonically in u, sigma > 0.5 <=> u > 1, and any
u within 3.8e-6 of threshold maps within 0.008 of sigma=0.5 -- far
inside one fp8 quantum (0.0625) -- so the host flags bytes near 0.5 as
suspects and recomputes those rows bit-exactly with the reference
formula (~1e-5 of rows).

Sharding: batch dim B=131072 split into 8 contiguous blocks of 16384
rows. Per core the block is laid out time-major as [128 partitions, 400
steps, 128 neurons] so each step is one [128,128] SBUF slice and DMA
chunks are per-partition contiguous.
"""

import os
import numpy as np

import concourse.bacc as bacc
import concourse.mybir as mybir
from concourse.tile import TileContext
from concourse.bass_utils import run_bass_kernel_spmd
from concourse.mybir import AluOpType as Op

B, L = 131072, 400
NCORES = 8
RPC = B // NCORES      # rows (neurons) per core
P = 128                # SBUF partitions
J = RPC // P           # neurons per partition = 128 (one step = [P, J] slice)

# Chunk schedule: geometric ramp so each chunk's input DMA lands just in
# time while DVE chews the previous ones; small tail chunks so the final
# ACT/DMA drain hides under compute. Sums to L.
CHUNKS = [4, 8, 20, 32, 56] + [64] * 3 + [40, 24, 16, 4, 4]
assert sum(CHUNKS) == L

G = int(os.environ.get("BASS_LIF_G", "2"))        # interleaved groups
PLANES = os.environ.get("BASS_LIF_PLANES", "sig8")
JG = J // G

DECAY_MUL95 = 0.95
DECAY_MUL05 = 0.05
TH = 1.0
EPS = 1e-4            # sign2 band half-width
KSIG = 8192.0         # sig8 sigmoid sharpness

_nc_cache = None
_lif_op = None


def _register_lif_op():
    """Register the fused LIF-step custom DVE op (idempotent):
    out = in0 + (in1 * (in1 < s1)) * s0
    """
    global _lif_op
    if _lif_op is not None:
        return _lif_op
    import concourse.dve_ops as dve_ops
    from concourse.dve_spec import Spec, Src0, Src1, C0, C1, lower
    from concourse.dve_spec import _has_src1
    from concourse.dve_uop import DveOpSpec

    name = "LIF_STEP_ANT"
    for op in dve_ops.OPS:
        if op.name == name:
            _lif_op = op
            return op
    body = Src0 + (Src1 * (Src1 < C1)) * C0
    spec = Spec(
        body=body,
        reference=lambda in0, in1, s0, s1, imm2: (
            in0 + (in1 * (in1 < s1).astype(np.float32)) * np.float32(s0)
        ).astype(np.float32),
    )
    sha = {}
    for ver in ("v3", "v4"):
        sha[ver] = DveOpSpec(
            name=name, opcode=0x1F, uops=lower(spec, ver=ver),
            rd1_en=_has_src1(spec),
        ).sha(ver)
    op = dve_ops.DveOp(name, spec, subdim=False, uops_sha=sha)
    dve_ops.OPS.append(op)
    dve_ops.CUSTOM_DVE_SPECS[name] = spec
    row = dve_ops._CUSTOM_DVE_ROW_BASE + len(dve_ops.OPS) - 1
    assert row < 0x20, "custom-DVE opcode rows exhausted"
    dve_ops._SUB_OPCODE_FOR_NAME[name] = row
    _lif_op = op
    return op


def _build():
    nc = bacc.Bacc(None, target_bir_lowering=False)
    X = nc.dram_tensor("X", [P, L * J], mybir.dt.float32, kind="ExternalInput")
    if PLANES == "sig8":
        S8 = nc.dram_tensor("S8", [P, L * J], mybir.dt.float8e4, kind="ExternalOutput")
    else:
        A = nc.dram_tensor("A", [P, L * J], mybir.dt.uint8, kind="ExternalOutput")
        Bp = nc.dram_tensor("Bq", [P, L * J], mybir.dt.uint8, kind="ExternalOutput")
    lif = _register_lif_op()

    with TileContext(nc) as tc:
        with (
            tc.tile_pool(name="state", bufs=1) as state_pool,
            tc.tile_pool(name="io", bufs=5) as io_pool,
            tc.tile_pool(name="pl", bufs=3) as pl_pool,
        ):
            if PLANES == "sig8":
                bsg = state_pool.tile([P, 1], mybir.dt.float32, name="bsg")
                nc.vector.memset(bsg[:], -KSIG)
            else:
                blo = state_pool.tile([P, 1], mybir.dt.float32, name="blo")
                bhi = state_pool.tile([P, 1], mybir.dt.float32, name="bhi")
                nc.vector.memset(blo[:], float(EPS - TH))
                nc.vector.memset(bhi[:], float(-EPS - TH))
            prev_tile, prev_t = None, 0
            t0 = 0
            for ch, TC in enumerate(CHUNKS):
                base = t0 * J
                t0 += TC
                xin = io_pool.tile([P, TC * J], mybir.dt.float32, name="xin")
                nc.sync.dma_start(xin[:], X[:, base : base + TC * J])
                for t in range(TC):
                    if prev_tile is None:
                        # v init is 0, so u_0 = I_0: the DMA'd input slice
                        # already is u_0 -- skip the first step's op.
                        prev_tile, prev_t = xin, 0
                        continue
                    sls = [
                        slice(t * J + g * JG, t * J + (g + 1) * JG) for g in range(G)
                    ]
                    psls = [
                        slice(prev_t * J + g * JG, prev_t * J + (g + 1) * JG)
                        for g in range(G)
                    ]
                    for g in range(G):
                        nc.vector._custom_dve(
                            lif,
                            out=xin[:, sls[g]],
                            in0=xin[:, sls[g]],
                            in1=prev_tile[:, psls[g]],
                            s0=DECAY_MUL95,
                            s1=TH,
                        )
                    prev_tile, prev_t = xin, t
                if PLANES == "sig8":
                    # sig = sigmoid(KSIG*(u-1)) -> fp8: monotone byte code of u;
                    # bytes near 0.5 (|u-1| <~ 4e-5) are the host-repair band.
                    ps = pl_pool.tile([P, TC * J], mybir.dt.float8e4, name="ps")
                    nc.scalar.activation(
                        ps[:], xin[:], mybir.ActivationFunctionType.Sigmoid,
                        bias=bsg[:], scale=KSIG,
                    )
                    nc.scalar.dma_start(S8[:, base : base + TC * J], ps[:])
                else:
                    pa = pl_pool.tile([P, TC * J], mybir.dt.uint8, name="pa")
                    pb = pl_pool.tile([P, TC * J], mybir.dt.uint8, name="pb")
                    nc.scalar.activation(
                        pa[:], xin[:], mybir.ActivationFunctionType.Sign,
                        bias=blo[:], scale=1.0,
                    )
                    nc.scalar.activation(
                        pb[:], xin[:], mybir.ActivationFunctionType.Sign,
                        bias=bhi[:], scale=1.0,
                    )
                    nc.scalar.dma_start(A[:, base : base + TC * J], pa[:])
                    nc.scalar.dma_start(Bp[:, base : base + TC * J], pb[:])
    nc.compile()
    return nc


def _get_nc():
    global _nc_cache
    if _nc_cache is None:
        _nc_cache = _build()
    return _nc_cache


def _shard(I):
    # Per-core host transposes run in parallel (numpy releases the GIL
    # during the strided copies).
    from concurrent.futures import ThreadPoolExecutor

    def one(c):
        Ic = I[c * RPC : (c + 1) * RPC]                    # [RPC, L]
        Xc = Ic.reshape(P, J, L).transpose(0, 2, 1)        # [P, L, J] time-major
        return {"X": np.ascontiguousarray(Xc).reshape(P, L * J)}

    with ThreadPoolExecutor(NCORES) as ex:
        return list(ex.map(one, range(NCORES)))


def _unshard_plane(results, key):
    from concurrent.futures import ThreadPoolExecutor

    out = np.empty((B, L), np.uint8)

    def one(c):
        r = np.asarray(results[c][key])
        if r.dtype != np.uint8:
            r = r.view(np.uint8) if r.dtype.itemsize == 1 else r.astype(np.uint8)
        Sc = r.reshape(P, L, J).transpose(0, 2, 1)         # [P, J, L]
        out[c * RPC : (c + 1) * RPC] = Sc.reshape(RPC, L)

    with ThreadPoolExecutor(NCORES) as ex:
        list(ex.map(one, range(NCORES)))
    return out


def _f8e4m3_lut():
    """byte -> float32 value of fp8 e4m3 (bias 7)."""
    b = np.arange(256, dtype=np.uint32)
    sign = np.where(b >> 7, -1.0, 1.0)
    e = (b >> 3) & 0xF
    m = b & 0x7
    val = np.where(
        e == 0,
        (m / 8.0) * 2.0 ** (-6),
        (1.0 + m / 8.0) * (2.0 ** (e.astype(np.int32) - 7)),
    )
    return (sign * val).astype(np.float32)


def _decode(I, results):
    f32 = np.float32
    if PLANES == "sig8":
        raw = _unshard_plane(results, "S8")
        val = _f8e4m3_lut()[raw]
        spikes = val > f32(0.5)
        suspect = np.abs(val - f32(0.5)) <= f32(0.05)
    else:
        pa = _unshard_plane(results, "A")
        pb = _unshard_plane(results, "Bq")
        spikes = pa == 1
        suspect = spikes & (pb != 1)
    rows = np.nonzero(suspect.any(axis=1))[0]
    out = spikes.astype(f32)
    if rows.size:
        # Bit-exact reference recurrence for the suspect rows only.
        Ir = I[rows]
        v = np.zeros(rows.size, f32)
        for t in range(L):
            u = ((v - v * f32(DECAY_MUL05)) + Ir[:, t]).astype(f32)
            s = u >= f32(TH)
            out[rows, t] = s
            v = np.where(s, f32(0.0), u)
    return out


def kernel(I, _trace=False):
    I = np.ascontiguousarray(np.asarray(I), dtype=np.float32)
    assert I.shape == (B, L), I.shape
    nc = _get_nc()
    br = run_bass_kernel_spmd(nc, _shard(I), core_ids=list(range(NCORES)), trace=_trace)
    out = _decode(I, br.results)
    if _trace:
        return out, br
    return out


# revision 27
# speedup vs baseline: 1.0535x; 1.0197x over previous
"""LIF neuron scan kernel for Trainium2, sharded over 8 NeuronCores.

Device recurrence, ONE custom DVE instruction per time step (f32):
    u_t = I_t + 0.95 * (u_{t-1} * (u_{t-1} < 1))
computed in-place over the input tile (state = previous u slice), via a
registered custom DVE op (4 ALU stages).  The mask-multiply is exact, so
this matches the fused form u = round(round(0.95*v)+I).

The fused decay differs from the reference's (v - v/20) by <= ~4e-6
over the whole trajectory (measured with synced resets), so the spike
raster can only flip where u lands within that distance of threshold.
The device emits one fp8-e4m3 plane sig = sigmoid(8192*(u-1)) (ACT
engine): bytes decode monotonically in u, sigma > 0.5 <=> u > 1, and any
u within 3.8e-6 of threshold maps within 0.008 of sigma=0.5 -- far
inside one fp8 quantum (0.0625) -- so the host flags bytes near 0.5 as
suspects and recomputes those rows bit-exactly with the reference
formula (~1e-5 of rows).

Sharding: batch dim B=131072 split into 8 contiguous blocks of 16384
rows. Per core the block is laid out time-major as [128 partitions, 400
steps, 128 neurons] so each step is one [128,128] SBUF slice and DMA
chunks are per-partition contiguous.
"""

import os
import numpy as np

import concourse.bacc as bacc
import concourse.mybir as mybir
from concourse.tile import TileContext
from concourse.bass_utils import run_bass_kernel_spmd
from concourse.mybir import AluOpType as Op

B, L = 131072, 400
NCORES = 8
RPC = B // NCORES      # rows (neurons) per core
P = 128                # SBUF partitions
J = RPC // P           # neurons per partition = 128 (one step = [P, J] slice)

# Chunk schedule: geometric ramp so each chunk's input DMA lands just in
# time while DVE chews the previous ones; small tail chunks so the final
# ACT/DMA drain hides under compute. Sums to L.
CHUNKS = [8, 16, 32] + [64] * 4 + [40, 24, 16, 4, 4]
assert sum(CHUNKS) == L

G = int(os.environ.get("BASS_LIF_G", "2"))        # interleaved groups
PLANES = os.environ.get("BASS_LIF_PLANES", "sig8")
JG = J // G

DECAY_MUL95 = 0.95
DECAY_MUL05 = 0.05
TH = 1.0
EPS = 1e-4            # sign2 band half-width
KSIG = 8192.0         # sig8 sigmoid sharpness

_nc_cache = None
_lif_op = None


def _register_lif_op():
    """Register the fused LIF-step custom DVE op (idempotent):
    out = in0 + (in1 * (in1 < s1)) * s0
    """
    global _lif_op
    if _lif_op is not None:
        return _lif_op
    import concourse.dve_ops as dve_ops
    from concourse.dve_spec import Spec, Src0, Src1, C0, C1, lower
    from concourse.dve_spec import _has_src1
    from concourse.dve_uop import DveOpSpec

    name = "LIF_STEP_ANT"
    for op in dve_ops.OPS:
        if op.name == name:
            _lif_op = op
            return op
    body = Src0 + (Src1 * (Src1 < C1)) * C0
    spec = Spec(
        body=body,
        reference=lambda in0, in1, s0, s1, imm2: (
            in0 + (in1 * (in1 < s1).astype(np.float32)) * np.float32(s0)
        ).astype(np.float32),
    )
    sha = {}
    for ver in ("v3", "v4"):
        sha[ver] = DveOpSpec(
            name=name, opcode=0x1F, uops=lower(spec, ver=ver),
            rd1_en=_has_src1(spec),
        ).sha(ver)
    op = dve_ops.DveOp(name, spec, subdim=False, uops_sha=sha)
    dve_ops.OPS.append(op)
    dve_ops.CUSTOM_DVE_SPECS[name] = spec
    row = dve_ops._CUSTOM_DVE_ROW_BASE + len(dve_ops.OPS) - 1
    assert row < 0x20, "custom-DVE opcode rows exhausted"
    dve_ops._SUB_OPCODE_FOR_NAME[name] = row
    _lif_op = op
    return op


def _build():
    nc = bacc.Bacc(None, target_bir_lowering=False)
    X = nc.dram_tensor("X", [P, L * J], mybir.dt.float32, kind="ExternalInput")
    if PLANES == "sig8":
        S8 = nc.dram_tensor("S8", [P, L * J], mybir.dt.float8e4, kind="ExternalOutput")
    else:
        A = nc.dram_tensor("A", [P, L * J], mybir.dt.uint8, kind="ExternalOutput")
        Bp = nc.dram_tensor("Bq", [P, L * J], mybir.dt.uint8, kind="ExternalOutput")
    lif = _register_lif_op()

    with TileContext(nc) as tc:
        with (
            tc.tile_pool(name="state", bufs=1) as state_pool,
            tc.tile_pool(name="io", bufs=5) as io_pool,
            tc.tile_pool(name="pl", bufs=3) as pl_pool,
        ):
            if PLANES == "sig8":
                bsg = state_pool.tile([P, 1], mybir.dt.float32, name="bsg")
                nc.vector.memset(bsg[:], -KSIG)
            else:
                blo = state_pool.tile([P, 1], mybir.dt.float32, name="blo")
                bhi = state_pool.tile([P, 1], mybir.dt.float32, name="bhi")
                nc.vector.memset(blo[:], float(EPS - TH))
                nc.vector.memset(bhi[:], float(-EPS - TH))
            prev_tile, prev_t = None, 0
            t0 = 0
            for ch, TC in enumerate(CHUNKS):
                base = t0 * J
                t0 += TC
                xin = io_pool.tile([P, TC * J], mybir.dt.float32, name="xin")
                nc.sync.dma_start(xin[:], X[:, base : base + TC * J])
                for t in range(TC):
                    if prev_tile is None:
                        # v init is 0, so u_0 = I_0: the DMA'd input slice
                        # already is u_0 -- skip the first step's op.
                        prev_tile, prev_t = xin, 0
                        continue
                    sls = [
                        slice(t * J + g * JG, t * J + (g + 1) * JG) for g in range(G)
                    ]
                    psls = [
                        slice(prev_t * J + g * JG, prev_t * J + (g + 1) * JG)
                        for g in range(G)
                    ]
                    for g in range(G):
                        nc.vector._custom_dve(
                            lif,
                            out=xin[:, sls[g]],
                            in0=xin[:, sls[g]],
                            in1=prev_tile[:, psls[g]],
                            s0=DECAY_MUL95,
                            s1=TH,
                        )
                    prev_tile, prev_t = xin, t
                if PLANES == "sig8":
                    # sig = sigmoid(KSIG*(u-1)) -> fp8: monotone byte code of u;
                    # bytes near 0.5 (|u-1| <~ 4e-5) are the host-repair band.
                    ps = pl_pool.tile([P, TC * J], mybir.dt.float8e4, name="ps")
                    nc.scalar.activation(
                        ps[:], xin[:], mybir.ActivationFunctionType.Sigmoid,
                        bias=bsg[:], scale=KSIG,
                    )
                    nc.scalar.dma_start(S8[:, base : base + TC * J], ps[:])
                else:
                    pa = pl_pool.tile([P, TC * J], mybir.dt.uint8, name="pa")
                    pb = pl_pool.tile([P, TC * J], mybir.dt.uint8, name="pb")
                    nc.scalar.activation(
                        pa[:], xin[:], mybir.ActivationFunctionType.Sign,
                        bias=blo[:], scale=1.0,
                    )
                    nc.scalar.activation(
                        pb[:], xin[:], mybir.ActivationFunctionType.Sign,
                        bias=bhi[:], scale=1.0,
                    )
                    nc.scalar.dma_start(A[:, base : base + TC * J], pa[:])
                    nc.scalar.dma_start(Bp[:, base : base + TC * J], pb[:])
    nc.compile()
    return nc


def _get_nc():
    global _nc_cache
    if _nc_cache is None:
        _nc_cache = _build()
    return _nc_cache


def _shard(I):
    # Per-core host transposes run in parallel (numpy releases the GIL
    # during the strided copies).
    from concurrent.futures import ThreadPoolExecutor

    def one(c):
        Ic = I[c * RPC : (c + 1) * RPC]                    # [RPC, L]
        Xc = Ic.reshape(P, J, L).transpose(0, 2, 1)        # [P, L, J] time-major
        return {"X": np.ascontiguousarray(Xc).reshape(P, L * J)}

    with ThreadPoolExecutor(NCORES) as ex:
        return list(ex.map(one, range(NCORES)))


def _unshard_plane(results, key):
    from concurrent.futures import ThreadPoolExecutor

    out = np.empty((B, L), np.uint8)

    def one(c):
        r = np.asarray(results[c][key])
        if r.dtype != np.uint8:
            r = r.view(np.uint8) if r.dtype.itemsize == 1 else r.astype(np.uint8)
        Sc = r.reshape(P, L, J).transpose(0, 2, 1)         # [P, J, L]
        out[c * RPC : (c + 1) * RPC] = Sc.reshape(RPC, L)

    with ThreadPoolExecutor(NCORES) as ex:
        list(ex.map(one, range(NCORES)))
    return out


def _f8e4m3_lut():
    """byte -> float32 value of fp8 e4m3 (bias 7)."""
    b = np.arange(256, dtype=np.uint32)
    sign = np.where(b >> 7, -1.0, 1.0)
    e = (b >> 3) & 0xF
    m = b & 0x7
    val = np.where(
        e == 0,
        (m / 8.0) * 2.0 ** (-6),
        (1.0 + m / 8.0) * (2.0 ** (e.astype(np.int32) - 7)),
    )
    return (sign * val).astype(np.float32)


def _decode(I, results):
    f32 = np.float32
    if PLANES == "sig8":
        raw = _unshard_plane(results, "S8")
        val = _f8e4m3_lut()[raw]
        spikes = val > f32(0.5)
        suspect = np.abs(val - f32(0.5)) <= f32(0.05)
    else:
        pa = _unshard_plane(results, "A")
        pb = _unshard_plane(results, "Bq")
        spikes = pa == 1
        suspect = spikes & (pb != 1)
    rows = np.nonzero(suspect.any(axis=1))[0]
    out = spikes.astype(f32)
    if rows.size:
        # Bit-exact reference recurrence for the suspect rows only.
        Ir = I[rows]
        v = np.zeros(rows.size, f32)
        for t in range(L):
            u = ((v - v * f32(DECAY_MUL05)) + Ir[:, t]).astype(f32)
            s = u >= f32(TH)
            out[rows, t] = s
            v = np.where(s, f32(0.0), u)
    return out


def kernel(I, _trace=False):
    I = np.ascontiguousarray(np.asarray(I), dtype=np.float32)
    assert I.shape == (B, L), I.shape
    nc = _get_nc()
    br = run_bass_kernel_spmd(nc, _shard(I), core_ids=list(range(NCORES)), trace=_trace)
    out = _decode(I, br.results)
    if _trace:
        return out, br
    return out


# revision 28
# speedup vs baseline: 1.0600x; 1.0062x over previous
"""LIF neuron scan kernel for Trainium2, sharded over 8 NeuronCores.

Device recurrence, ONE custom DVE instruction per time step (f32):
    u_t = I_t + 0.95 * (u_{t-1} * (u_{t-1} < 1))
computed in-place over the input tile (state = previous u slice), via a
registered custom DVE op (4 ALU stages).  The mask-multiply is exact, so
this matches the fused form u = round(round(0.95*v)+I).

The fused decay differs from the reference's (v - v/20) by <= ~4e-6
over the whole trajectory (measured with synced resets), so the spike
raster can only flip where u lands within that distance of threshold.
The device emits one fp8-e4m3 plane sig = sigmoid(8192*(u-1)) (ACT
engine): bytes decode monotonically in u, sigma > 0.5 <=> u > 1, and any
u within 3.8e-6 of threshold maps within 0.008 of sigma=0.5 -- far
inside one fp8 quantum (0.0625) -- so the host flags bytes near 0.5 as
suspects and recomputes those rows bit-exactly with the reference
formula (~1e-5 of rows).

Sharding: batch dim B=131072 split into 8 contiguous blocks of 16384
rows. Per core the block is laid out time-major as [128 partitions, 400
steps, 128 neurons] so each step is one [128,128] SBUF slice and DMA
chunks are per-partition contiguous.
"""

import os
import numpy as np

import concourse.bacc as bacc
import concourse.mybir as mybir
from concourse.tile import TileContext
from concourse.bass_utils import run_bass_kernel_spmd
from concourse.mybir import AluOpType as Op

B, L = 131072, 400
NCORES = 8
RPC = B // NCORES      # rows (neurons) per core
P = 128                # SBUF partitions
J = RPC // P           # neurons per partition = 128 (one step = [P, J] slice)

# Chunk schedule: geometric ramp so each chunk's input DMA lands just in
# time while DVE chews the previous ones; small tail chunks so the final
# ACT/DMA drain hides under compute. Sums to L.
CHUNKS = [8, 16, 32] + [64] * 4 + [40, 24, 16, 4, 4]
assert sum(CHUNKS) == L

G = int(os.environ.get("BASS_LIF_G", "2"))        # interleaved groups
PLANES = os.environ.get("BASS_LIF_PLANES", "sig8")
JG = J // G

DECAY_MUL95 = 0.95
DECAY_MUL05 = 0.05
TH = 1.0
EPS = 1e-4            # sign2 band half-width
KSIG = 8192.0         # sig8 sigmoid sharpness

_nc_cache = None
_lif_op = None


def _register_lif_op():
    """Register the fused LIF-step custom DVE op (idempotent):
    out = in0 + (in1 * (in1 < s1)) * s0
    """
    global _lif_op
    if _lif_op is not None:
        return _lif_op
    import concourse.dve_ops as dve_ops
    from concourse.dve_spec import Spec, Src0, Src1, C0, C1, lower
    from concourse.dve_spec import _has_src1
    from concourse.dve_uop import DveOpSpec

    name = "LIF_STEP_ANT"
    for op in dve_ops.OPS:
        if op.name == name:
            _lif_op = op
            return op
    body = Src0 + (Src1 * (Src1 < C1)) * C0
    spec = Spec(
        body=body,
        reference=lambda in0, in1, s0, s1, imm2: (
            in0 + (in1 * (in1 < s1).astype(np.float32)) * np.float32(s0)
        ).astype(np.float32),
    )
    sha = {}
    for ver in ("v3", "v4"):
        sha[ver] = DveOpSpec(
            name=name, opcode=0x1F, uops=lower(spec, ver=ver),
            rd1_en=_has_src1(spec),
        ).sha(ver)
    op = dve_ops.DveOp(name, spec, subdim=False, uops_sha=sha)
    dve_ops.OPS.append(op)
    dve_ops.CUSTOM_DVE_SPECS[name] = spec
    row = dve_ops._CUSTOM_DVE_ROW_BASE + len(dve_ops.OPS) - 1
    assert row < 0x20, "custom-DVE opcode rows exhausted"
    dve_ops._SUB_OPCODE_FOR_NAME[name] = row
    _lif_op = op
    return op


def _build():
    nc = bacc.Bacc(None, target_bir_lowering=False)
    X = nc.dram_tensor("X", [P, L * J], mybir.dt.float32, kind="ExternalInput")
    if PLANES == "sig8":
        S8 = nc.dram_tensor("S8", [P, L * J], mybir.dt.float8e4, kind="ExternalOutput")
    else:
        A = nc.dram_tensor("A", [P, L * J], mybir.dt.uint8, kind="ExternalOutput")
        Bp = nc.dram_tensor("Bq", [P, L * J], mybir.dt.uint8, kind="ExternalOutput")
    lif = _register_lif_op()

    with TileContext(nc) as tc:
        with (
            tc.tile_pool(name="state", bufs=1) as state_pool,
            tc.tile_pool(name="io", bufs=5) as io_pool,
            tc.tile_pool(name="pl", bufs=3) as pl_pool,
        ):
            if PLANES == "sig8":
                bsg = state_pool.tile([P, 1], mybir.dt.float32, name="bsg")
                nc.vector.memset(bsg[:], -KSIG)
            else:
                blo = state_pool.tile([P, 1], mybir.dt.float32, name="blo")
                bhi = state_pool.tile([P, 1], mybir.dt.float32, name="bhi")
                nc.vector.memset(blo[:], float(EPS - TH))
                nc.vector.memset(bhi[:], float(-EPS - TH))
            prev_tile, prev_t = None, 0
            t0 = 0
            for ch, TC in enumerate(CHUNKS):
                base = t0 * J
                t0 += TC
                xin = io_pool.tile([P, TC * J], mybir.dt.float32, name="xin")
                if ch == 0:
                    # Split the first chunk's load so the opening LIF ops wait
                    # only on a 2-step sliver, not the whole chunk transfer.
                    nc.sync.dma_start(xin[:, : 2 * J], X[:, : 2 * J])
                    nc.sync.dma_start(xin[:, 2 * J :], X[:, 2 * J : TC * J])
                else:
                    nc.sync.dma_start(xin[:], X[:, base : base + TC * J])
                for t in range(TC):
                    if prev_tile is None:
                        # v init is 0, so u_0 = I_0: the DMA'd input slice
                        # already is u_0 -- skip the first step's op.
                        prev_tile, prev_t = xin, 0
                        continue
                    sls = [
                        slice(t * J + g * JG, t * J + (g + 1) * JG) for g in range(G)
                    ]
                    psls = [
                        slice(prev_t * J + g * JG, prev_t * J + (g + 1) * JG)
                        for g in range(G)
                    ]
                    for g in range(G):
                        nc.vector._custom_dve(
                            lif,
                            out=xin[:, sls[g]],
                            in0=xin[:, sls[g]],
                            in1=prev_tile[:, psls[g]],
                            s0=DECAY_MUL95,
                            s1=TH,
                        )
                    prev_tile, prev_t = xin, t
                if PLANES == "sig8":
                    # sig = sigmoid(KSIG*(u-1)) -> fp8: monotone byte code of u;
                    # bytes near 0.5 (|u-1| <~ 4e-5) are the host-repair band.
                    ps = pl_pool.tile([P, TC * J], mybir.dt.float8e4, name="ps")
                    nc.scalar.activation(
                        ps[:], xin[:], mybir.ActivationFunctionType.Sigmoid,
                        bias=bsg[:], scale=KSIG,
                    )
                    nc.scalar.dma_start(S8[:, base : base + TC * J], ps[:])
                else:
                    pa = pl_pool.tile([P, TC * J], mybir.dt.uint8, name="pa")
                    pb = pl_pool.tile([P, TC * J], mybir.dt.uint8, name="pb")
                    nc.scalar.activation(
                        pa[:], xin[:], mybir.ActivationFunctionType.Sign,
                        bias=blo[:], scale=1.0,
                    )
                    nc.scalar.activation(
                        pb[:], xin[:], mybir.ActivationFunctionType.Sign,
                        bias=bhi[:], scale=1.0,
                    )
                    nc.scalar.dma_start(A[:, base : base + TC * J], pa[:])
                    nc.scalar.dma_start(Bp[:, base : base + TC * J], pb[:])
    nc.compile()
    return nc


def _get_nc():
    global _nc_cache
    if _nc_cache is None:
        _nc_cache = _build()
    return _nc_cache


def _shard(I):
    # Per-core host transposes run in parallel (numpy releases the GIL
    # during the strided copies).
    from concurrent.futures import ThreadPoolExecutor

    def one(c):
        Ic = I[c * RPC : (c + 1) * RPC]                    # [RPC, L]
        Xc = Ic.reshape(P, J, L).transpose(0, 2, 1)        # [P, L, J] time-major
        return {"X": np.ascontiguousarray(Xc).reshape(P, L * J)}

    with ThreadPoolExecutor(NCORES) as ex:
        return list(ex.map(one, range(NCORES)))


def _unshard_plane(results, key):
    from concurrent.futures import ThreadPoolExecutor

    out = np.empty((B, L), np.uint8)

    def one(c):
        r = np.asarray(results[c][key])
        if r.dtype != np.uint8:
            r = r.view(np.uint8) if r.dtype.itemsize == 1 else r.astype(np.uint8)
        Sc = r.reshape(P, L, J).transpose(0, 2, 1)         # [P, J, L]
        out[c * RPC : (c + 1) * RPC] = Sc.reshape(RPC, L)

    with ThreadPoolExecutor(NCORES) as ex:
        list(ex.map(one, range(NCORES)))
    return out


def _f8e4m3_lut():
    """byte -> float32 value of fp8 e4m3 (bias 7)."""
    b = np.arange(256, dtype=np.uint32)
    sign = np.where(b >> 7, -1.0, 1.0)
    e = (b >> 3) & 0xF
    m = b & 0x7
    val = np.where(
        e == 0,
        (m / 8.0) * 2.0 ** (-6),
        (1.0 + m / 8.0) * (2.0 ** (e.astype(np.int32) - 7)),
    )
    return (sign * val).astype(np.float32)


def _decode(I, results):
    f32 = np.float32
    if PLANES == "sig8":
        raw = _unshard_plane(results, "S8")
        val = _f8e4m3_lut()[raw]
        spikes = val > f32(0.5)
        suspect = np.abs(val - f32(0.5)) <= f32(0.05)
    else:
        pa = _unshard_plane(results, "A")
        pb = _unshard_plane(results, "Bq")
        spikes = pa == 1
        suspect = spikes & (pb != 1)
    rows = np.nonzero(suspect.any(axis=1))[0]
    out = spikes.astype(f32)
    if rows.size:
        # Bit-exact reference recurrence for the suspect rows only.
        Ir = I[rows]
        v = np.zeros(rows.size, f32)
        for t in range(L):
            u = ((v - v * f32(DECAY_MUL05)) + Ir[:, t]).astype(f32)
            s = u >= f32(TH)
            out[rows, t] = s
            v = np.where(s, f32(0.0), u)
    return out


def kernel(I, _trace=False):
    I = np.ascontiguousarray(np.asarray(I), dtype=np.float32)
    assert I.shape == (B, L), I.shape
    nc = _get_nc()
    br = run_bass_kernel_spmd(nc, _shard(I), core_ids=list(range(NCORES)), trace=_trace)
    out = _decode(I, br.results)
    if _trace:
        return out, br
    return out
